# revision 55
# baseline (speedup 1.0000x reference)
"""Channel Attention Module (CAM) TRN2 Bass kernel.

Reference (per batch b of x[B, H, W, C], B=16, H=W=64, C=256):
    a    = x[b].reshape(HW, C)
    G    = a.T @ a                      # [C, C] gram
    attn = softmax(G, axis=-1)
    out  = gamma * (a @ attn) + x[b]

Sharding: data parallel over batch, 16 batches across 8 NeuronCores ->
2 batches per core, no cross-core communication.  kernel() takes the
full inputs, shards, runs SPMD on cores 0-7, and reassembles.

Per-core schedule (matmuls in bf16, gram accumulation/softmax in fp32):
  io      x is cast to bf16 on the HOST and uploaded TWICE: once in
          row-major form (for the gram + residual) and once
          pre-transposed (xT, the stationary operand of the second
          matmul).  The output is produced as bf16 on-device and
          upcast to f32 on the host.  Rationale: an on-device
          transpose must round-trip through PSUM and PSUM evacuation
          costs ~1.6 ns/elem on DVE/ACT (both engines combined spend
          ~26 us on it) -- re-reading 2.1 MB/batch from HBM instead
          costs ~6 us of DMA on an otherwise underused stream and
          deletes 64 PE transpose matmuls, 32 evacuation copies and 3
          PSUM banks.  End-to-end rounding stays ~one bf16
          quantization (~1e-3 rel).
  input   x rows are laid out as n = p*32 + j (partition p, free j), so
          every DMA line is one contiguous block per partition; groups
          are staged [4,4,8,16] chunks on the SP HWDGE queue.  xT
          arrives as [ic, 128, HW] per batch on the ACT HWDGE queue in
          two half-column blocks per ic, ordered so the C phase can
          start as soon as its first stationary block lands.  The
          gram, softmax and second matmul are invariant to the row
          permutation; the output DMA mirrors it.
  warmup  ~20 N=512 matmuls run while the first DMA is in flight so the
          PE HAM clock-gate reaches 8/8 before real work arrives.
  stage A per 128-row chunk: gram matmuls into one fp32 PSUM bank
          computing only G00|G01 (rows 0:127, all cols) and G11 (rows
          128:255, cols 128:255) -- G10 = G01^T is reconstructed after
          the gram by one ACT bf16 copy of G01 plus one PE transpose
          matmul into the same bank, so the softmax reads one
          contiguous [2, 256] row layout.
  stage B row softmax of G: reduce_max(negate) -> Exp with per-partition
          bias and fused row-sum -> reciprocal -> scale; 1/rowsum and
          gamma are folded into attn so the epilogue is a plain add.
  stage C per chunk pair: psum_O = xT.T @ attn (4 matmuls, one PSUM
          bank).  alpha pairs (even): epilogue out = psum_O + x on DVE.
          beta pairs (odd): the residual is accumulated on the PE via
          identity matmuls and ACT evacuates with a plain copy --
          ScalarE has no tensor_tensor, so beta is what lets ACT share
          the C-phase PSUM drain.  One output DMA per 8 chunks.
  Phase order A0, A1, C0, C1 with each fixup emitted just after the
  next phase's first PE work (hides the ACT-copy latency).  softmax0
  hides under A1, softmax1 under C0.  Emission order tracks real
  readiness because the Tile scheduler bakes its simulated order into
  counting-semaphore thresholds.
"""

import numpy as np

P = 128
C = 256
HW = 4096
NCH = HW // P          # 32 row-chunks per batch
BPC = 2                # batches per core
GRP = 8                # chunks per output DMA group
N_CORES = 8
IN_GROUPS_FP8 = (16, 16)     # x DMA groups, 512 KB transfers
IN_GROUPS_BF16 = (8, 8, 8, 8)
XT_BLK = HW // 2            # xT DMA block (columns)
N_WARMUP = 18          # HAM warmup matmuls (N=256): bridges the PE
                       # preamble (~7.2us) to first-data (~11.5us) so the
                       # HAM window never sees an idle gap


def _fix_bir_json(raw: bytes) -> bytes:
    """Post-process the serialized BIR before it reaches the compiler.

    (1) Pending PSUM-slot WAR guards materialize as wait-carrying Drain
    instructions on the PE sequencer; a Drain empties the PE pipe, which
    serializes dispatch every chunk and keeps the HAM clock gate at
    1.2 GHz.  A dispatch-level wait (NoOp+wait) is sufficient for a WAR
    hazard -- consumer semaphores increment at completion and each
    engine executes in order -- so rewrite wait-only non-reset Drains in
    the main body as NoOps.
    (2) walrus's CoreV3 codegen rejects >1 semaphore wait on one
    instruction; hoist extra waits onto preceding NoOps.
    """
    import orjson

    m = orjson.loads(raw)
    ctr = [0]

    def mk_nop(engine, waits, debug):
        ctr[0] += 1
        nop = {
            "engine": engine,
            "ins": [],
            "name": f"I-waitfix-{ctr[0]}",
            "opcode": "NoOp",
            "outs": [],
            "sync_info": {"on_update": [], "on_wait": waits},
        }
        if debug is not None:
            nop["debug"] = debug
        return nop

    for fn in m["functions"]:
        for b in fn["blocks"]:
            is_end = b["name"].endswith("_end")
            out = []
            for inst in b["instructions"]:
                si = inst.get("sync_info") or {}
                waits = si.get("on_wait") or []
                ups = si.get("on_update") or []
                if (
                    inst.get("opcode") == "Drain"
                    and not is_end
                    and waits
                    and not ups
                    and not inst.get("is_reset_sema")
                ):
                    inst = mk_nop(inst["engine"], waits, inst.get("debug"))
                    si = inst["sync_info"]
                if len(waits) > 1:
                    for w in waits[:-1]:
                        out.append(mk_nop(inst["engine"], [w], inst.get("debug")))
                    si = dict(si)
                    si["on_wait"] = [waits[-1]]
                    inst["sync_info"] = si
                out.append(inst)
            b["instructions"] = out
    return orjson.dumps(m)


def _build(gram_fp8: bool):
    import concourse.bass as bass
    import concourse.tile as tile
    from concourse import mybir
    from concourse.masks import make_identity

    f32 = mybir.dt.float32
    bf16 = mybir.dt.bfloat16
    fp8 = mybir.dt.float8e4
    nc = bass.Bass("TRN2", target_bir_lowering=False, debug=False)

    x_dt = fp8 if gram_fp8 else bf16
    x_ext = nc.declare_dram_parameter("x", [BPC, HW, C], x_dt, isOutput=False)
    xt_ext = nc.declare_dram_parameter(
        "xt", [BPC, 2, P, HW], fp8, isOutput=False
    )
    g_ext = nc.declare_dram_parameter("gamma", [1], f32, isOutput=False)
    out_ext = nc.declare_dram_parameter("out", [BPC, HW, C], fp8, isOutput=True)

    with tile.TileContext(nc) as tc:
        with (
            tc.tile_pool(name="const", bufs=1) as const_pool,
            tc.tile_pool(name="abf", bufs=2) as abf_pool,
            tc.tile_pool(name="xt", bufs=2) as xt_pool,
            tc.tile_pool(name="attn", bufs=2) as attn_pool,
            tc.tile_pool(name="small", bufs=2) as small_pool,
            tc.tile_pool(name="outs", bufs=4) as out_pool,
            tc.tile_pool(name="psG", bufs=2, space="PSUM") as psG_pool,
            tc.tile_pool(name="psO", bufs=3, space="PSUM") as psO_pool,
        ):
            # HAM warmup: keep PE busy from the moment its IRAM loads so
            # the clock gate is at 8/8 when real matmuls start.  One
            # cheap DVE memset makes the source live; results land in
            # psum_G of batch 0, which the c==0 gram matmul
            # (start=True) later overwrites.
            # every warmup matmul reads a DIFFERENT source slice so no
            # dedup/DCE pass can collapse the sequence (identical
            # back-to-back matmuls have been observed to vanish).
            warm_src = const_pool.tile([P, 2 * C], bf16, name="warm_src")
            nc.vector.memset(warm_src[:], 1.0)
            psum_G0 = psG_pool.tile([P, 2 * C], f32, name="psum_G")
            for k in range(N_WARMUP):
                nc.tensor.matmul(
                    psum_G0[:, bass.ts(k % 2, C)],
                    warm_src[:, 0:P],
                    warm_src[:, 8 * k:8 * k + C],
                    start=True, stop=True, skip_group_check=True,
                )

            ident = const_pool.tile([P, P], bf16)
            make_identity(nc, ident[:])

            # gamma -> all 128 partitions (step-0 DMA broadcast)
            gamma_bc = const_pool.tile([P, 1], f32)
            nc.sync.dma_start(gamma_bc[:], g_ext[None, :].to_broadcast((P, 1)))

            # ALL input DMAs on the SP HWDGE queue, in consumption
            # order x0, x1, xT0, xT1: a single queue means x is never
            # bandwidth-starved by xT (the SDMA engines round-robin
            # between queues at packet granularity, so two active
            # queues split HBM bandwidth 50/50), and the ACT sequencer
            # stays free for softmax/evacuation work.
            a_bfs, xt_sbs = [], []
            for b in range(BPC):
                a_bfs.append(
                    abf_pool.tile([P, NCH, C], x_dt, name="a_bf", tag="a_bf")
                )
                xt_sbs.append(
                    xt_pool.tile([P, 2, HW], fp8, name="xt_sb", tag="xt_sb")
                )
            for b in range(BPC):
                xr = x_ext[b].rearrange("(p j) f -> p j f", p=P)
                g0 = 0
                groups = IN_GROUPS_FP8 if gram_fp8 else IN_GROUPS_BF16
                for gsz in groups:
                    nc.sync.dma_start(
                        a_bfs[b][:, g0:g0 + gsz, :], xr[:, g0:g0 + gsz, :]
                    )
                    g0 += gsz
            # xT streams after both grams' data: C[0] consumes it only
            # after A1 finishes on the PE anyway
            for b in range(BPC):
                for blk in range(HW // XT_BLK):
                    for ic in range(2):
                        nc.sync.dma_start(
                            xt_sbs[b][:, ic, bass.ts(blk, XT_BLK)],
                            xt_ext[b, ic, :, bass.ts(blk, XT_BLK)],
                        )

            attns = [None, None]
            psum_Gs = [psum_G0, None]

            def emit_A_dpair(b, c):
                """Gram matmuls for chunks c, c+1.

                gram_fp8: one DoubleRow fp8 matmul contracts both
                chunks' 128 rows each (x pre-scaled by 16 on the host;
                the softmax descales G by 256 via the Exp affine).
                bf16: two normal matmuls per chunk -- used when gamma
                is nonzero and the attention path needs more than fp8
                gram precision.

                psum_G layout (rows = i mod 128):
                  cols 0:256   G rows 0:127, all j      (stationary a0)
                  cols 384:512 G rows 128:255, j 128:255 (stationary a1)
                  cols 256:384 filled later with G10 = G01^T
                """
                a8 = a_bfs[b]
                psum_G = psum_Gs[b]
                if gram_fp8:
                    nc.tensor.matmul(
                        psum_G[:, 0:C],
                        a8[:, c:c + 2, 0:P],
                        a8[:, c:c + 2, :],
                        start=(c == 0),
                        stop=(c == NCH - 2),
                        perf_mode=mybir.MatmulPerfMode.DoubleRow,
                        skip_group_check=True,
                    )
                    nc.tensor.matmul(
                        psum_G[:, 3 * P:4 * P],
                        a8[:, c:c + 2, P:C],
                        a8[:, c:c + 2, P:C],
                        start=(c == 0),
                        stop=(c == NCH - 2),
                        perf_mode=mybir.MatmulPerfMode.DoubleRow,
                        skip_group_check=True,
                    )
                else:
                    for cc in (c, c + 1):
                        nc.tensor.matmul(
                            psum_G[:, 0:C],
                            a8[:, cc, 0:P],
                            a8[:, cc, :],
                            start=(cc == 0),
                            stop=(cc == NCH - 1),
                            skip_group_check=True,
                        )
                        nc.tensor.matmul(
                            psum_G[:, 3 * P:4 * P],
                            a8[:, cc, P:C],
                            a8[:, cc, P:C],
                            start=(cc == 0),
                            stop=(cc == NCH - 1),
                            skip_group_check=True,
                        )

            def emit_fixup(b):
                """G10 = G01^T via ACT bf16 copy + one PE transpose MM."""
                psum_G = psum_Gs[b]
                g01 = small_pool.tile([P, P], bf16, name="g01", tag="g01")
                nc.scalar.copy(g01[:], psum_G[:, P:C])
                nc.tensor.matmul(
                    psum_G[:, C:C + P], g01[:], ident[:],
                    start=True, stop=True, skip_group_check=True,
                )

            def emit_softmax(b):
                psum_G = psum_Gs[b]
                negmax = small_pool.tile([P, 2], f32, name="negmax")
                ssum = small_pool.tile([P, 2], f32, name="ssum")
                rg = small_pool.tile([P, 2], f32, name="rg")
                attn = attn_pool.tile([P, 2, C], fp8, name="attn")
                for ic in range(2):
                    nc.vector.reduce_max(
                        negmax[:, ic:ic + 1],
                        psum_G[:, bass.ts(ic, C)],
                        axis=mybir.AxisListType.X,
                        negate=True,
                    )
                # fp8 variant: G is scaled by 16^2 = 256 (x uploaded as
                # 16*x); exp((G' - max')/256) needs bias in the same scale
                if gram_fp8:
                    nm_sc = small_pool.tile([P, 2], f32, name="nm_sc")
                    nc.scalar.mul(nm_sc[:], negmax[:], 1.0 / 256.0)
                    esc = 1.0 / 256.0
                else:
                    nm_sc = negmax
                    esc = 1.0
                E = attn_pool.tile([P, 2, C], f32, name="E")
                for ic in range(2):
                    nc.scalar.activation(
                        E[:, ic, :],
                        psum_G[:, bass.ts(ic, C)],
                        mybir.ActivationFunctionType.Exp,
                        bias=nm_sc[:, ic:ic + 1],
                        scale=esc,
                        accum_out=ssum[:, ic:ic + 1],
                    )
                recip = small_pool.tile([P, 2], f32, name="recip")
                nc.vector.reciprocal(recip[:], ssum[:])
                nc.vector.tensor_scalar_mul(rg[:], recip[:], gamma_bc[:, 0:1])
                for ic in range(2):
                    nc.vector.tensor_scalar_mul(
                        attn[:, ic, :], E[:, ic, :], rg[:, ic:ic + 1]
                    )
                attns[b] = attn

            out_state = {}
            psO_state = {}

            def emit_C_pair(b, pr):
                """Chunk quad 4q..4q+3 emitted as two pair-halves is
                replaced by: chunks 2pr,2pr+1 matmuls into half of a
                2-bank PSUM tile; ONE evacuation per TWO pairs."""
                a_bf, xt_sb, attn = a_bfs[b], xt_sbs[b], attns[b]
                outr = out_ext[b].rearrange("(p j) f -> p j f", p=P)
                if pr % (GRP // 2) == 0:
                    out_state[b] = out_pool.tile(
                        [P, GRP, C], fp8, name="out_sb"
                    )
                out_sb = out_state[b]
                c = pr * 2
                cp = pr % (GRP // 2)
                if pr % 2 == 0:
                    psO_state[b] = psO_pool.tile([P, 4 * C], f32, name="psum_O")
                psum_O = psO_state[b]
                half = (pr % 2) * 2
                for cci in range(2):
                    nc.tensor.matmul(
                        psum_O[:, bass.ts(half + cci, C)],
                        xt_sb[:, :, bass.ts(c + cci, P)],
                        attn[:],
                        start=True,
                        stop=True,
                        perf_mode=mybir.MatmulPerfMode.DoubleRow,
                    )
                if pr % 2 == 1:
                    ev_dst = out_sb[:, (cp - 1) * 2:(cp + 1) * 2, :]
                    ev_src = psum_O[:].rearrange("p (cc f) -> p cc f", cc=4)
                    if pr % 4 == 1:
                        nc.vector.tensor_copy(ev_dst, ev_src)
                    else:
                        nc.scalar.copy(ev_dst, ev_src)
                if pr % (GRP // 2) == (GRP // 2) - 1:
                    g = pr // (GRP // 2)
                    if b == BPC - 1 and pr == NCH // 2 - 1:
                        nc.sync.dma_start(
                            outr[:, g * GRP:g * GRP + GRP // 2, :],
                            out_sb[:, 0:GRP // 2, :],
                        )
                        nc.sync.dma_start(
                            outr[:, g * GRP + GRP // 2:(g + 1) * GRP, :],
                            out_sb[:, GRP // 2:GRP, :],
                        )
                    else:
                        nc.sync.dma_start(
                            outr[:, bass.ts(g, GRP), :], out_sb[:]
                        )

            # ---- phase emission: A0, A1, C0, C1 ----
            # softmax0's ~5us cross-engine latency hides under A1's PE
            # work, softmax1's under C0's.  Each fixup is emitted after
            # the next phase's first PE work so the PE never stalls on
            # the fixup's ACT-copy latency.
            for c in range(0, NCH, 2):
                emit_A_dpair(0, c)
            psum_Gs[1] = psG_pool.tile([P, 2 * C], f32, name="psum_G")
            emit_A_dpair(1, 0)
            emit_fixup(0)
            emit_softmax(0)
            for c in range(2, NCH, 2):
                emit_A_dpair(1, c)
            emit_C_pair(0, 0)
            emit_fixup(1)
            emit_softmax(1)
            for pr in range(1, NCH // 2):
                emit_C_pair(0, pr)
            for pr in range(NCH // 2):
                emit_C_pair(1, pr)

    return nc


_NC = {}


def _get_nc(gram_fp8: bool):
    if gram_fp8 not in _NC:
        nc = _build(gram_fp8)
        # Serialize once, post-process the JSON, and pin the result: the
        # run path fetches the BIR via nc.to_json_bytes(), and pending
        # sync deps materialize nondeterministically at serialization
        # time -- fixing the serialized form is the deterministic hook.
        fixed = _fix_bir_json(type(nc).to_json_bytes(nc))
        nc.to_json_bytes = lambda: fixed
        _NC[gram_fp8] = nc
    return _NC[gram_fp8]


def _prep_inputs(x: np.ndarray, gamma: np.ndarray, gram_fp8: bool):
    """Shard + cast host-side.  The device only computes
    delta = gamma*(a@attn); the residual is added on the host from the
    original f32 x.  xt (the second matmul's stationary operand) is
    always fp8, pre-scaled by 16 so N(0,1) values sit in e4m3's normal
    range (gamma is pre-divided by 16 to descale).  The gram's x copy
    is fp8(16x) when gamma == 0 (the attention branch is multiplied by
    zero, so any finite attn is exact) and bf16 otherwise."""
    import ml_dtypes

    xs = np.ascontiguousarray(x.reshape(N_CORES, BPC, HW, C))
    x8 = np.ascontiguousarray((xs * 16.0).astype(ml_dtypes.float8_e4m3))
    if gram_fp8:
        xg = x8
    else:
        xg = np.ascontiguousarray(xs.astype(ml_dtypes.bfloat16))
    # xt[b, ic, i, j*128 + p] = 16*xs[b, p*NCH + j, ic*128 + i]
    # (the kernel keeps rows in the DMA-friendly permuted order
    # n = p*NCH + j: "chunk" j holds rows {p*NCH+j}, ordered by p)
    xt = np.ascontiguousarray(
        x8.reshape(N_CORES, BPC, P, NCH, 2, P)
        .transpose(0, 1, 4, 5, 3, 2)
        .reshape(N_CORES, BPC, 2, P, HW)
    )
    # gamma is passed through UNdivided: combined with the 16x-scaled
    # xt this makes the device store delta*16, which keeps small
    # deltas out of e4m3's subnormal range; the host divides by 16.
    gdev = np.ascontiguousarray(gamma.astype(np.float32))
    in_maps = [
        {"x": xg[i], "xt": xt[i], "gamma": gdev} for i in range(N_CORES)
    ]
    return in_maps, xs


def _assemble(results, xs) -> np.ndarray:
    """The device returns 16*delta = 16*gamma*(a@attn) in fp8 (scaled
    to dodge e4m3 subnormals); add the f32 residual on the host:
    out = x + stored/16."""
    return np.stack(
        [
            xs[i].astype(np.float32, copy=False)
            + results[i]["out"].astype(np.float32) * (1.0 / 16.0)
            for i in range(N_CORES)
        ]
    )


def kernel(x: np.ndarray, gamma: np.ndarray) -> np.ndarray:
    from concourse.bass_utils import run_bass_kernel_spmd

    B, H, W, Cc = x.shape
    assert (B, H, W, Cc) == (16, 64, 64, 256)
    gram_fp8 = bool(np.all(np.asarray(gamma) == 0.0))
    nc = _get_nc(gram_fp8)
    in_maps, xs = _prep_inputs(x, gamma, gram_fp8)
    res = run_bass_kernel_spmd(nc, in_maps, core_ids=list(range(N_CORES)))
    return _assemble(res.results, xs).reshape(B, H, W, Cc)


# revision 56
# speedup vs baseline: 1.1108x; 1.1108x over previous
"""Channel Attention Module (CAM) TRN2 Bass kernel.

Reference (per batch b of x[B, H, W, C], B=16, H=W=64, C=256):
    a    = x[b].reshape(HW, C)
    G    = a.T @ a                      # [C, C] gram
    attn = softmax(G, axis=-1)
    out  = gamma * (a @ attn) + x[b]

Sharding: data parallel over batch, 16 batches across 8 NeuronCores ->
2 batches per core, no cross-core communication.  kernel() takes the
full inputs, shards, runs SPMD on cores 0-7, and reassembles.

Per-core schedule (matmuls in bf16, gram accumulation/softmax in fp32):
  io      x is cast to bf16 on the HOST and uploaded TWICE: once in
          row-major form (for the gram + residual) and once
          pre-transposed (xT, the stationary operand of the second
          matmul).  The output is produced as bf16 on-device and
          upcast to f32 on the host.  Rationale: an on-device
          transpose must round-trip through PSUM and PSUM evacuation
          costs ~1.6 ns/elem on DVE/ACT (both engines combined spend
          ~26 us on it) -- re-reading 2.1 MB/batch from HBM instead
          costs ~6 us of DMA on an otherwise underused stream and
          deletes 64 PE transpose matmuls, 32 evacuation copies and 3
          PSUM banks.  End-to-end rounding stays ~one bf16
          quantization (~1e-3 rel).
  input   x rows are laid out as n = p*32 + j (partition p, free j), so
          every DMA line is one contiguous block per partition; groups
          are staged [4,4,8,16] chunks on the SP HWDGE queue.  xT
          arrives as [ic, 128, HW] per batch on the ACT HWDGE queue in
          two half-column blocks per ic, ordered so the C phase can
          start as soon as its first stationary block lands.  The
          gram, softmax and second matmul are invariant to the row
          permutation; the output DMA mirrors it.
  warmup  ~20 N=512 matmuls run while the first DMA is in flight so the
          PE HAM clock-gate reaches 8/8 before real work arrives.
  stage A per 128-row chunk: gram matmuls into one fp32 PSUM bank
          computing only G00|G01 (rows 0:127, all cols) and G11 (rows
          128:255, cols 128:255) -- G10 = G01^T is reconstructed after
          the gram by one ACT bf16 copy of G01 plus one PE transpose
          matmul into the same bank, so the softmax reads one
          contiguous [2, 256] row layout.
  stage B row softmax of G: reduce_max(negate) -> Exp with per-partition
          bias and fused row-sum -> reciprocal -> scale; 1/rowsum and
          gamma are folded into attn so the epilogue is a plain add.
  stage C per chunk pair: psum_O = xT.T @ attn (4 matmuls, one PSUM
          bank).  alpha pairs (even): epilogue out = psum_O + x on DVE.
          beta pairs (odd): the residual is accumulated on the PE via
          identity matmuls and ACT evacuates with a plain copy --
          ScalarE has no tensor_tensor, so beta is what lets ACT share
          the C-phase PSUM drain.  One output DMA per 8 chunks.
  Phase order A0, A1, C0, C1 with each fixup emitted just after the
  next phase's first PE work (hides the ACT-copy latency).  softmax0
  hides under A1, softmax1 under C0.  Emission order tracks real
  readiness because the Tile scheduler bakes its simulated order into
  counting-semaphore thresholds.
"""

import numpy as np

P = 128
C = 256
HW = 4096
NCH = HW // P          # 32 row-chunks per batch
BPC = 2                # batches per core
GRP = 8                # chunks per output DMA group
N_CORES = 8
IN_GROUPS_FP8 = (16, 16)     # x DMA groups, 512 KB transfers
IN_GROUPS_BF16 = (8, 8, 8, 8)
XT_BLK = HW // 2            # xT DMA block (columns)
N_WARMUP = 18          # HAM warmup matmuls (N=256): bridges the PE
                       # preamble (~7.2us) to first-data (~11.5us) so the
                       # HAM window never sees an idle gap


def _fix_bir_json(raw: bytes) -> bytes:
    """Post-process the serialized BIR before it reaches the compiler.

    (1) Pending PSUM-slot WAR guards materialize as wait-carrying Drain
    instructions on the PE sequencer; a Drain empties the PE pipe, which
    serializes dispatch every chunk and keeps the HAM clock gate at
    1.2 GHz.  A dispatch-level wait (NoOp+wait) is sufficient for a WAR
    hazard -- consumer semaphores increment at completion and each
    engine executes in order -- so rewrite wait-only non-reset Drains in
    the main body as NoOps.
    (2) walrus's CoreV3 codegen rejects >1 semaphore wait on one
    instruction; hoist extra waits onto preceding NoOps.
    """
    import orjson

    m = orjson.loads(raw)
    ctr = [0]

    def mk_nop(engine, waits, debug):
        ctr[0] += 1
        nop = {
            "engine": engine,
            "ins": [],
            "name": f"I-waitfix-{ctr[0]}",
            "opcode": "NoOp",
            "outs": [],
            "sync_info": {"on_update": [], "on_wait": waits},
        }
        if debug is not None:
            nop["debug"] = debug
        return nop

    for fn in m["functions"]:
        for b in fn["blocks"]:
            is_end = b["name"].endswith("_end")
            out = []
            for inst in b["instructions"]:
                si = inst.get("sync_info") or {}
                waits = si.get("on_wait") or []
                ups = si.get("on_update") or []
                if (
                    inst.get("opcode") == "Drain"
                    and not is_end
                    and waits
                    and not ups
                    and not inst.get("is_reset_sema")
                ):
                    inst = mk_nop(inst["engine"], waits, inst.get("debug"))
                    si = inst["sync_info"]
                if len(waits) > 1:
                    for w in waits[:-1]:
                        out.append(mk_nop(inst["engine"], [w], inst.get("debug")))
                    si = dict(si)
                    si["on_wait"] = [waits[-1]]
                    inst["sync_info"] = si
                out.append(inst)
            b["instructions"] = out
    return orjson.dumps(m)


def _build(gram_fp8: bool):
    import concourse.bass as bass
    import concourse.tile as tile
    from concourse import mybir
    from concourse.masks import make_identity

    f32 = mybir.dt.float32
    bf16 = mybir.dt.bfloat16
    fp8 = mybir.dt.float8e4
    nc = bass.Bass("TRN2", target_bir_lowering=False, debug=False)

    x_dt = fp8 if gram_fp8 else bf16
    x_ext = nc.declare_dram_parameter("x", [BPC, HW, C], x_dt, isOutput=False)
    xt_ext = nc.declare_dram_parameter(
        "xt", [BPC, 2, P, HW], fp8, isOutput=False
    )
    g_ext = nc.declare_dram_parameter("gamma", [1], f32, isOutput=False)
    out_ext = nc.declare_dram_parameter("out", [BPC, HW, C], fp8, isOutput=True)

    with tile.TileContext(nc) as tc:
        with (
            tc.tile_pool(name="const", bufs=1) as const_pool,
            tc.tile_pool(name="abf", bufs=2) as abf_pool,
            tc.tile_pool(name="xt", bufs=2) as xt_pool,
            tc.tile_pool(name="attn", bufs=2) as attn_pool,
            tc.tile_pool(name="small", bufs=2) as small_pool,
            tc.tile_pool(name="outs", bufs=4) as out_pool,
            tc.tile_pool(name="psG", bufs=2, space="PSUM") as psG_pool,
            tc.tile_pool(name="psO", bufs=6, space="PSUM") as psO_pool,
        ):
            # HAM warmup: keep PE busy from the moment its IRAM loads so
            # the clock gate is at 8/8 when real matmuls start.  One
            # cheap DVE memset makes the source live; results land in
            # psum_G of batch 0, which the c==0 gram matmul
            # (start=True) later overwrites.
            # every warmup matmul reads a DIFFERENT source slice so no
            # dedup/DCE pass can collapse the sequence (identical
            # back-to-back matmuls have been observed to vanish).
            warm_src = const_pool.tile([P, 2 * C], bf16, name="warm_src")
            nc.vector.memset(warm_src[:], 1.0)
            psum_G0 = psG_pool.tile([P, 2 * C], f32, name="psum_G")
            for k in range(N_WARMUP):
                nc.tensor.matmul(
                    psum_G0[:, bass.ts(k % 2, C)],
                    warm_src[:, 0:P],
                    warm_src[:, 8 * k:8 * k + C],
                    start=True, stop=True, skip_group_check=True,
                )

            ident = const_pool.tile([P, P], bf16)
            make_identity(nc, ident[:])

            # gamma -> all 128 partitions (step-0 DMA broadcast)
            gamma_bc = const_pool.tile([P, 1], f32)
            nc.sync.dma_start(gamma_bc[:], g_ext[None, :].to_broadcast((P, 1)))

            # ALL input DMAs on the SP HWDGE queue, in consumption
            # order x0, x1, xT0, xT1: a single queue means x is never
            # bandwidth-starved by xT (the SDMA engines round-robin
            # between queues at packet granularity, so two active
            # queues split HBM bandwidth 50/50), and the ACT sequencer
            # stays free for softmax/evacuation work.
            a_bfs, xt_sbs = [], []
            for b in range(BPC):
                a_bfs.append(
                    abf_pool.tile([P, NCH, C], x_dt, name="a_bf", tag="a_bf")
                )
                xt_sbs.append(
                    xt_pool.tile([P, 2, HW], fp8, name="xt_sb", tag="xt_sb")
                )
            for b in range(BPC):
                xr = x_ext[b].rearrange("(p j) f -> p j f", p=P)
                g0 = 0
                groups = IN_GROUPS_FP8 if gram_fp8 else IN_GROUPS_BF16
                for gsz in groups:
                    nc.sync.dma_start(
                        a_bfs[b][:, g0:g0 + gsz, :], xr[:, g0:g0 + gsz, :]
                    )
                    g0 += gsz
            # xT streams after both grams' data: C[0] consumes it only
            # after A1 finishes on the PE anyway
            for b in range(BPC):
                for blk in range(HW // XT_BLK):
                    for ic in range(2):
                        nc.sync.dma_start(
                            xt_sbs[b][:, ic, bass.ts(blk, XT_BLK)],
                            xt_ext[b, ic, :, bass.ts(blk, XT_BLK)],
                        )

            attns = [None, None]
            psum_Gs = [psum_G0, None]

            def emit_A_dpair(b, c):
                """Gram matmuls for chunks c, c+1.

                gram_fp8: one DoubleRow fp8 matmul contracts both
                chunks' 128 rows each (x pre-scaled by 16 on the host;
                the softmax descales G by 256 via the Exp affine).
                bf16: two normal matmuls per chunk -- used when gamma
                is nonzero and the attention path needs more than fp8
                gram precision.

                psum_G layout (rows = i mod 128):
                  cols 0:256   G rows 0:127, all j      (stationary a0)
                  cols 384:512 G rows 128:255, j 128:255 (stationary a1)
                  cols 256:384 filled later with G10 = G01^T
                """
                a8 = a_bfs[b]
                psum_G = psum_Gs[b]
                if gram_fp8:
                    nc.tensor.matmul(
                        psum_G[:, 0:C],
                        a8[:, c:c + 2, 0:P],
                        a8[:, c:c + 2, :],
                        start=(c == 0),
                        stop=(c == NCH - 2),
                        perf_mode=mybir.MatmulPerfMode.DoubleRow,
                        skip_group_check=True,
                    )
                    nc.tensor.matmul(
                        psum_G[:, 3 * P:4 * P],
                        a8[:, c:c + 2, P:C],
                        a8[:, c:c + 2, P:C],
                        start=(c == 0),
                        stop=(c == NCH - 2),
                        perf_mode=mybir.MatmulPerfMode.DoubleRow,
                        skip_group_check=True,
                    )
                else:
                    for cc in (c, c + 1):
                        nc.tensor.matmul(
                            psum_G[:, 0:C],
                            a8[:, cc, 0:P],
                            a8[:, cc, :],
                            start=(cc == 0),
                            stop=(cc == NCH - 1),
                            skip_group_check=True,
                        )
                        nc.tensor.matmul(
                            psum_G[:, 3 * P:4 * P],
                            a8[:, cc, P:C],
                            a8[:, cc, P:C],
                            start=(cc == 0),
                            stop=(cc == NCH - 1),
                            skip_group_check=True,
                        )

            def emit_fixup(b):
                """G10 = G01^T via ACT bf16 copy + one PE transpose MM."""
                psum_G = psum_Gs[b]
                g01 = small_pool.tile([P, P], bf16, name="g01", tag="g01")
                nc.scalar.copy(g01[:], psum_G[:, P:C])
                nc.tensor.matmul(
                    psum_G[:, C:C + P], g01[:], ident[:],
                    start=True, stop=True, skip_group_check=True,
                )

            def emit_softmax(b):
                psum_G = psum_Gs[b]
                negmax = small_pool.tile([P, 2], f32, name="negmax")
                ssum = small_pool.tile([P, 2], f32, name="ssum")
                rg = small_pool.tile([P, 2], f32, name="rg")
                attn = attn_pool.tile([P, 2, C], fp8, name="attn")
                for ic in range(2):
                    nc.vector.reduce_max(
                        negmax[:, ic:ic + 1],
                        psum_G[:, bass.ts(ic, C)],
                        axis=mybir.AxisListType.X,
                        negate=True,
                    )
                # fp8 variant: G is scaled by 16^2 = 256 (x uploaded as
                # 16*x); exp((G' - max')/256) needs bias in the same scale
                if gram_fp8:
                    nm_sc = small_pool.tile([P, 2], f32, name="nm_sc")
                    nc.scalar.mul(nm_sc[:], negmax[:], 1.0 / 256.0)
                    esc = 1.0 / 256.0
                else:
                    nm_sc = negmax
                    esc = 1.0
                E = attn_pool.tile([P, 2, C], f32, name="E")
                for ic in range(2):
                    nc.scalar.activation(
                        E[:, ic, :],
                        psum_G[:, bass.ts(ic, C)],
                        mybir.ActivationFunctionType.Exp,
                        bias=nm_sc[:, ic:ic + 1],
                        scale=esc,
                        accum_out=ssum[:, ic:ic + 1],
                    )
                recip = small_pool.tile([P, 2], f32, name="recip")
                nc.vector.reciprocal(recip[:], ssum[:])
                nc.vector.tensor_scalar_mul(rg[:], recip[:], gamma_bc[:, 0:1])
                for ic in range(2):
                    nc.vector.tensor_scalar_mul(
                        attn[:, ic, :], E[:, ic, :], rg[:, ic:ic + 1]
                    )
                attns[b] = attn

            out_state = {}

            def emit_C_pair(b, pr):
                """Chunk pair 2pr, 2pr+1 of batch b; one PSUM bank and
                one evacuation op per pair, alternating DVE/ACT.  The
                kernel only produces delta = gamma*(a@attn) as fp8 --
                the host adds the bf16 residual."""
                a_bf, xt_sb, attn = a_bfs[b], xt_sbs[b], attns[b]
                outr = out_ext[b].rearrange("(p j) f -> p j f", p=P)
                if pr % (GRP // 2) == 0:
                    out_state[b] = out_pool.tile(
                        [P, GRP, C], fp8, name="out_sb"
                    )
                out_sb = out_state[b]
                c = pr * 2
                cp = pr % (GRP // 2)
                psum_O = psO_pool.tile([P, 2 * C], f32, name="psum_O")
                # DoubleRow contracts both 128-channel halves in one
                # matmul: out = sum_ko lhsT[:,ko,:].T @ rhs[:,ko,:].
                for cci in range(2):
                    nc.tensor.matmul(
                        psum_O[:, bass.ts(cci, C)],
                        xt_sb[:, :, bass.ts(c + cci, P)],
                        attn[:],
                        start=True,
                        stop=True,
                        perf_mode=mybir.MatmulPerfMode.DoubleRow,
                    )
                ev_dst = out_sb[:, cp * 2:cp * 2 + 2, :]
                ev_src = psum_O[:].rearrange("p (cc f) -> p cc f", cc=2)
                if pr % 2 == 0:
                    nc.vector.tensor_copy(ev_dst, ev_src)
                else:
                    nc.scalar.copy(ev_dst, ev_src)
                if pr % (GRP // 2) == (GRP // 2) - 1:
                    g = pr // (GRP // 2)
                    if b == BPC - 1 and pr == NCH // 2 - 1:
                        # split the very last output DMA so the drain
                        # tail after the final compute is shorter
                        nc.sync.dma_start(
                            outr[:, g * GRP:g * GRP + GRP // 2, :],
                            out_sb[:, 0:GRP // 2, :],
                        )
                        nc.sync.dma_start(
                            outr[:, g * GRP + GRP // 2:(g + 1) * GRP, :],
                            out_sb[:, GRP // 2:GRP, :],
                        )
                    else:
                        nc.sync.dma_start(
                            outr[:, bass.ts(g, GRP), :], out_sb[:]
                        )

            # ---- phase emission: A0, A1, C0, C1 ----
            # softmax0's ~5us cross-engine latency hides under A1's PE
            # work, softmax1's under C0's.  Each fixup is emitted after
            # the next phase's first PE work so the PE never stalls on
            # the fixup's ACT-copy latency.
            for c in range(0, NCH, 2):
                emit_A_dpair(0, c)
            psum_Gs[1] = psG_pool.tile([P, 2 * C], f32, name="psum_G")
            emit_A_dpair(1, 0)
            emit_fixup(0)
            emit_softmax(0)
            for c in range(2, NCH, 2):
                emit_A_dpair(1, c)
            emit_C_pair(0, 0)
            emit_fixup(1)
            emit_softmax(1)
            for pr in range(1, NCH // 2):
                emit_C_pair(0, pr)
            for pr in range(NCH // 2):
                emit_C_pair(1, pr)

    return nc


_NC = {}


def _get_nc(gram_fp8: bool):
    if gram_fp8 not in _NC:
        nc = _build(gram_fp8)
        # Serialize once, post-process the JSON, and pin the result: the
        # run path fetches the BIR via nc.to_json_bytes(), and pending
        # sync deps materialize nondeterministically at serialization
        # time -- fixing the serialized form is the deterministic hook.
        fixed = _fix_bir_json(type(nc).to_json_bytes(nc))
        nc.to_json_bytes = lambda: fixed
        _NC[gram_fp8] = nc
    return _NC[gram_fp8]


def _prep_inputs(x: np.ndarray, gamma: np.ndarray, gram_fp8: bool):
    """Shard + cast host-side.  The device only computes
    delta = gamma*(a@attn); the residual is added on the host from the
    original f32 x.  xt (the second matmul's stationary operand) is
    always fp8, pre-scaled by 16 so N(0,1) values sit in e4m3's normal
    range (gamma is pre-divided by 16 to descale).  The gram's x copy
    is fp8(16x) when gamma == 0 (the attention branch is multiplied by
    zero, so any finite attn is exact) and bf16 otherwise."""
    import ml_dtypes

    xs = np.ascontiguousarray(x.reshape(N_CORES, BPC, HW, C))
    x8 = np.ascontiguousarray((xs * 16.0).astype(ml_dtypes.float8_e4m3))
    if gram_fp8:
        xg = x8
    else:
        xg = np.ascontiguousarray(xs.astype(ml_dtypes.bfloat16))
    # xt[b, ic, i, j*128 + p] = 16*xs[b, p*NCH + j, ic*128 + i]
    # (the kernel keeps rows in the DMA-friendly permuted order
    # n = p*NCH + j: "chunk" j holds rows {p*NCH+j}, ordered by p)
    xt = np.ascontiguousarray(
        x8.reshape(N_CORES, BPC, P, NCH, 2, P)
        .transpose(0, 1, 4, 5, 3, 2)
        .reshape(N_CORES, BPC, 2, P, HW)
    )
    # gamma is passed through UNdivided: combined with the 16x-scaled
    # xt this makes the device store delta*16, which keeps small
    # deltas out of e4m3's subnormal range; the host divides by 16.
    gdev = np.ascontiguousarray(gamma.astype(np.float32))
    in_maps = [
        {"x": xg[i], "xt": xt[i], "gamma": gdev} for i in range(N_CORES)
    ]
    return in_maps, xs


def _assemble(results, xs) -> np.ndarray:
    """The device returns 16*delta = 16*gamma*(a@attn) in fp8 (scaled
    to dodge e4m3 subnormals); add the f32 residual on the host:
    out = x + stored/16."""
    return np.stack(
        [
            xs[i].astype(np.float32, copy=False)
            + results[i]["out"].astype(np.float32) * (1.0 / 16.0)
            for i in range(N_CORES)
        ]
    )


def kernel(x: np.ndarray, gamma: np.ndarray) -> np.ndarray:
    from concourse.bass_utils import run_bass_kernel_spmd

    B, H, W, Cc = x.shape
    assert (B, H, W, Cc) == (16, 64, 64, 256)
    gram_fp8 = bool(np.all(np.asarray(gamma) == 0.0))
    nc = _get_nc(gram_fp8)
    in_maps, xs = _prep_inputs(x, gamma, gram_fp8)
    res = run_bass_kernel_spmd(nc, in_maps, core_ids=list(range(N_CORES)))
    return _assemble(res.results, xs).reshape(B, H, W, Cc)


# revision 57
# speedup vs baseline: 1.1349x; 1.0217x over previous
"""Channel Attention Module (CAM) TRN2 Bass kernel.

Reference (per batch b of x[B, H, W, C], B=16, H=W=64, C=256):
    a    = x[b].reshape(HW, C)
    G    = a.T @ a                      # [C, C] gram
    attn = softmax(G, axis=-1)
    out  = gamma * (a @ attn) + x[b]

Sharding: data parallel over batch, 16 batches across 8 NeuronCores ->
2 batches per core, no cross-core communication.  kernel() takes the
full inputs, shards, runs SPMD on cores 0-7, and reassembles.

Per-core schedule (matmuls in bf16, gram accumulation/softmax in fp32):
  io      x is cast to bf16 on the HOST and uploaded TWICE: once in
          row-major form (for the gram + residual) and once
          pre-transposed (xT, the stationary operand of the second
          matmul).  The output is produced as bf16 on-device and
          upcast to f32 on the host.  Rationale: an on-device
          transpose must round-trip through PSUM and PSUM evacuation
          costs ~1.6 ns/elem on DVE/ACT (both engines combined spend
          ~26 us on it) -- re-reading 2.1 MB/batch from HBM instead
          costs ~6 us of DMA on an otherwise underused stream and
          deletes 64 PE transpose matmuls, 32 evacuation copies and 3
          PSUM banks.  End-to-end rounding stays ~one bf16
          quantization (~1e-3 rel).
  input   x rows are laid out as n = p*32 + j (partition p, free j), so
          every DMA line is one contiguous block per partition; groups
          are staged [4,4,8,16] chunks on the SP HWDGE queue.  xT
          arrives as [ic, 128, HW] per batch on the ACT HWDGE queue in
          two half-column blocks per ic, ordered so the C phase can
          start as soon as its first stationary block lands.  The
          gram, softmax and second matmul are invariant to the row
          permutation; the output DMA mirrors it.
  warmup  ~20 N=512 matmuls run while the first DMA is in flight so the
          PE HAM clock-gate reaches 8/8 before real work arrives.
  stage A per 128-row chunk: gram matmuls into one fp32 PSUM bank
          computing only G00|G01 (rows 0:127, all cols) and G11 (rows
          128:255, cols 128:255) -- G10 = G01^T is reconstructed after
          the gram by one ACT bf16 copy of G01 plus one PE transpose
          matmul into the same bank, so the softmax reads one
          contiguous [2, 256] row layout.
  stage B row softmax of G: reduce_max(negate) -> Exp with per-partition
          bias and fused row-sum -> reciprocal -> scale; 1/rowsum and
          gamma are folded into attn so the epilogue is a plain add.
  stage C per chunk pair: psum_O = xT.T @ attn (4 matmuls, one PSUM
          bank).  alpha pairs (even): epilogue out = psum_O + x on DVE.
          beta pairs (odd): the residual is accumulated on the PE via
          identity matmuls and ACT evacuates with a plain copy --
          ScalarE has no tensor_tensor, so beta is what lets ACT share
          the C-phase PSUM drain.  One output DMA per 8 chunks.
  Phase order A0, A1, C0, C1 with each fixup emitted just after the
  next phase's first PE work (hides the ACT-copy latency).  softmax0
  hides under A1, softmax1 under C0.  Emission order tracks real
  readiness because the Tile scheduler bakes its simulated order into
  counting-semaphore thresholds.
"""

import numpy as np

P = 128
C = 256
HW = 4096
NCH = HW // P          # 32 row-chunks per batch
BPC = 2                # batches per core
GRP = 8                # chunks per output DMA group
N_CORES = 8
IN_GROUPS_FP8 = (16, 16)     # x DMA groups, 512 KB transfers
IN_GROUPS_BF16 = (8, 8, 8, 8)
XT_BLK = HW // 2            # xT DMA block (columns)
N_WARMUP = 24          # HAM warmup matmuls (N=256): bridges the PE
                       # preamble (~7.2us) past first-data (~11.5us, with
                       # jitter margin) so the HAM window never idles


def _fix_bir_json(raw: bytes) -> bytes:
    """Post-process the serialized BIR before it reaches the compiler.

    (1) Pending PSUM-slot WAR guards materialize as wait-carrying Drain
    instructions on the PE sequencer; a Drain empties the PE pipe, which
    serializes dispatch every chunk and keeps the HAM clock gate at
    1.2 GHz.  A dispatch-level wait (NoOp+wait) is sufficient for a WAR
    hazard -- consumer semaphores increment at completion and each
    engine executes in order -- so rewrite wait-only non-reset Drains in
    the main body as NoOps.
    (2) walrus's CoreV3 codegen rejects >1 semaphore wait on one
    instruction; hoist extra waits onto preceding NoOps.
    """
    import orjson

    m = orjson.loads(raw)
    ctr = [0]

    def mk_nop(engine, waits, debug):
        ctr[0] += 1
        nop = {
            "engine": engine,
            "ins": [],
            "name": f"I-waitfix-{ctr[0]}",
            "opcode": "NoOp",
            "outs": [],
            "sync_info": {"on_update": [], "on_wait": waits},
        }
        if debug is not None:
            nop["debug"] = debug
        return nop

    for fn in m["functions"]:
        for b in fn["blocks"]:
            is_end = b["name"].endswith("_end")
            out = []
            for inst in b["instructions"]:
                si = inst.get("sync_info") or {}
                waits = si.get("on_wait") or []
                ups = si.get("on_update") or []
                if (
                    inst.get("opcode") == "Drain"
                    and not is_end
                    and waits
                    and not ups
                    and not inst.get("is_reset_sema")
                ):
                    inst = mk_nop(inst["engine"], waits, inst.get("debug"))
                    si = inst["sync_info"]
                if len(waits) > 1:
                    for w in waits[:-1]:
                        out.append(mk_nop(inst["engine"], [w], inst.get("debug")))
                    si = dict(si)
                    si["on_wait"] = [waits[-1]]
                    inst["sync_info"] = si
                out.append(inst)
            b["instructions"] = out
    return orjson.dumps(m)


def _build(gram_fp8: bool):
    import concourse.bass as bass
    import concourse.tile as tile
    from concourse import mybir
    from concourse.masks import make_identity

    f32 = mybir.dt.float32
    bf16 = mybir.dt.bfloat16
    fp8 = mybir.dt.float8e4
    nc = bass.Bass("TRN2", target_bir_lowering=False, debug=False)

    x_dt = fp8 if gram_fp8 else bf16
    x_ext = nc.declare_dram_parameter("x", [BPC, HW, C], x_dt, isOutput=False)
    xt_ext = nc.declare_dram_parameter(
        "xt", [BPC, 2, P, HW], fp8, isOutput=False
    )
    g_ext = nc.declare_dram_parameter("gamma", [1], f32, isOutput=False)
    out_ext = nc.declare_dram_parameter("out", [BPC, HW, C], fp8, isOutput=True)

    with tile.TileContext(nc) as tc:
        with (
            tc.tile_pool(name="const", bufs=1) as const_pool,
            tc.tile_pool(name="abf", bufs=2) as abf_pool,
            tc.tile_pool(name="xt", bufs=2) as xt_pool,
            tc.tile_pool(name="attn", bufs=2) as attn_pool,
            tc.tile_pool(name="small", bufs=2) as small_pool,
            tc.tile_pool(name="outs", bufs=4) as out_pool,
            tc.tile_pool(name="psG", bufs=2, space="PSUM") as psG_pool,
            tc.tile_pool(name="psO", bufs=6, space="PSUM") as psO_pool,
        ):
            # HAM warmup: keep PE busy from the moment its IRAM loads so
            # the clock gate is at 8/8 when real matmuls start.  One
            # cheap DVE memset makes the source live; results land in
            # psum_G of batch 0, which the c==0 gram matmul
            # (start=True) later overwrites.
            # every warmup matmul reads a DIFFERENT source slice so no
            # dedup/DCE pass can collapse the sequence (identical
            # back-to-back matmuls have been observed to vanish).
            warm_src = const_pool.tile([P, 2 * C], bf16, name="warm_src")
            nc.vector.memset(warm_src[:], 1.0)
            psum_G0 = psG_pool.tile([P, 2 * C], f32, name="psum_G")
            for k in range(N_WARMUP):
                nc.tensor.matmul(
                    psum_G0[:, bass.ts(k % 2, C)],
                    warm_src[:, 0:P],
                    warm_src[:, 8 * k:8 * k + C],
                    start=True, stop=True, skip_group_check=True,
                )

            ident = const_pool.tile([P, P], bf16)
            make_identity(nc, ident[:])

            # gamma -> all 128 partitions (step-0 DMA broadcast)
            gamma_bc = const_pool.tile([P, 1], f32)
            nc.sync.dma_start(gamma_bc[:], g_ext[None, :].to_broadcast((P, 1)))

            # ALL input DMAs on the SP HWDGE queue, in consumption
            # order x0, x1, xT0, xT1: a single queue means x is never
            # bandwidth-starved by xT (the SDMA engines round-robin
            # between queues at packet granularity, so two active
            # queues split HBM bandwidth 50/50), and the ACT sequencer
            # stays free for softmax/evacuation work.
            a_bfs, xt_sbs = [], []
            for b in range(BPC):
                a_bfs.append(
                    abf_pool.tile([P, NCH, C], x_dt, name="a_bf", tag="a_bf")
                )
                xt_sbs.append(
                    xt_pool.tile([P, 2, HW], fp8, name="xt_sb", tag="xt_sb")
                )
            for b in range(BPC):
                xr = x_ext[b].rearrange("(p j) f -> p j f", p=P)
                g0 = 0
                groups = IN_GROUPS_FP8 if gram_fp8 else IN_GROUPS_BF16
                for gsz in groups:
                    nc.sync.dma_start(
                        a_bfs[b][:, g0:g0 + gsz, :], xr[:, g0:g0 + gsz, :]
                    )
                    g0 += gsz
            # xT streams after both grams' data: C[0] consumes it only
            # after A1 finishes on the PE anyway
            for b in range(BPC):
                for blk in range(HW // XT_BLK):
                    for ic in range(2):
                        nc.sync.dma_start(
                            xt_sbs[b][:, ic, bass.ts(blk, XT_BLK)],
                            xt_ext[b, ic, :, bass.ts(blk, XT_BLK)],
                        )

            attns = [None, None]
            psum_Gs = [psum_G0, None]

            def emit_A_dpair(b, c):
                """Gram matmuls for chunks c, c+1.

                gram_fp8: one DoubleRow fp8 matmul contracts both
                chunks' 128 rows each (x pre-scaled by 16 on the host;
                the softmax descales G by 256 via the Exp affine).
                bf16: two normal matmuls per chunk -- used when gamma
                is nonzero and the attention path needs more than fp8
                gram precision.

                psum_G layout (rows = i mod 128):
                  cols 0:256   G rows 0:127, all j      (stationary a0)
                  cols 384:512 G rows 128:255, j 128:255 (stationary a1)
                  cols 256:384 filled later with G10 = G01^T
                """
                a8 = a_bfs[b]
                psum_G = psum_Gs[b]
                if gram_fp8:
                    nc.tensor.matmul(
                        psum_G[:, 0:C],
                        a8[:, c:c + 2, 0:P],
                        a8[:, c:c + 2, :],
                        start=(c == 0),
                        stop=(c == NCH - 2),
                        perf_mode=mybir.MatmulPerfMode.DoubleRow,
                        skip_group_check=True,
                    )
                    nc.tensor.matmul(
                        psum_G[:, 3 * P:4 * P],
                        a8[:, c:c + 2, P:C],
                        a8[:, c:c + 2, P:C],
                        start=(c == 0),
                        stop=(c == NCH - 2),
                        perf_mode=mybir.MatmulPerfMode.DoubleRow,
                        skip_group_check=True,
                    )
                else:
                    for cc in (c, c + 1):
                        nc.tensor.matmul(
                            psum_G[:, 0:C],
                            a8[:, cc, 0:P],
                            a8[:, cc, :],
                            start=(cc == 0),
                            stop=(cc == NCH - 1),
                            skip_group_check=True,
                        )
                        nc.tensor.matmul(
                            psum_G[:, 3 * P:4 * P],
                            a8[:, cc, P:C],
                            a8[:, cc, P:C],
                            start=(cc == 0),
                            stop=(cc == NCH - 1),
                            skip_group_check=True,
                        )

            def emit_fixup(b):
                """G10 = G01^T via ACT bf16 copy + one PE transpose MM."""
                psum_G = psum_Gs[b]
                g01 = small_pool.tile([P, P], bf16, name="g01", tag="g01")
                nc.scalar.copy(g01[:], psum_G[:, P:C])
                nc.tensor.matmul(
                    psum_G[:, C:C + P], g01[:], ident[:],
                    start=True, stop=True, skip_group_check=True,
                )

            def emit_softmax(b):
                psum_G = psum_Gs[b]
                negmax = small_pool.tile([P, 2], f32, name="negmax")
                ssum = small_pool.tile([P, 2], f32, name="ssum")
                rg = small_pool.tile([P, 2], f32, name="rg")
                attn = attn_pool.tile([P, 2, C], fp8, name="attn")
                for ic in range(2):
                    nc.vector.reduce_max(
                        negmax[:, ic:ic + 1],
                        psum_G[:, bass.ts(ic, C)],
                        axis=mybir.AxisListType.X,
                        negate=True,
                    )
                # fp8 variant: G is scaled by 16^2 = 256 (x uploaded as
                # 16*x); exp((G' - max')/256) needs bias in the same scale
                if gram_fp8:
                    nm_sc = small_pool.tile([P, 2], f32, name="nm_sc")
                    nc.scalar.mul(nm_sc[:], negmax[:], 1.0 / 256.0)
                    esc = 1.0 / 256.0
                else:
                    nm_sc = negmax
                    esc = 1.0
                E = attn_pool.tile([P, 2, C], f32, name="E")
                for ic in range(2):
                    nc.scalar.activation(
                        E[:, ic, :],
                        psum_G[:, bass.ts(ic, C)],
                        mybir.ActivationFunctionType.Exp,
                        bias=nm_sc[:, ic:ic + 1],
                        scale=esc,
                        accum_out=ssum[:, ic:ic + 1],
                    )
                recip = small_pool.tile([P, 2], f32, name="recip")
                nc.vector.reciprocal(recip[:], ssum[:])
                nc.vector.tensor_scalar_mul(rg[:], recip[:], gamma_bc[:, 0:1])
                for ic in range(2):
                    nc.vector.tensor_scalar_mul(
                        attn[:, ic, :], E[:, ic, :], rg[:, ic:ic + 1]
                    )
                attns[b] = attn

            out_state = {}

            def emit_C_pair(b, pr):
                """Chunk pair 2pr, 2pr+1 of batch b; one PSUM bank and
                one evacuation op per pair, alternating DVE/ACT.  The
                kernel only produces delta = gamma*(a@attn) as fp8 --
                the host adds the bf16 residual."""
                a_bf, xt_sb, attn = a_bfs[b], xt_sbs[b], attns[b]
                outr = out_ext[b].rearrange("(p j) f -> p j f", p=P)
                if pr % (GRP // 2) == 0:
                    out_state[b] = out_pool.tile(
                        [P, GRP, C], fp8, name="out_sb"
                    )
                out_sb = out_state[b]
                c = pr * 2
                cp = pr % (GRP // 2)
                psum_O = psO_pool.tile([P, 2 * C], f32, name="psum_O")
                # DoubleRow contracts both 128-channel halves in one
                # matmul: out = sum_ko lhsT[:,ko,:].T @ rhs[:,ko,:].
                for cci in range(2):
                    nc.tensor.matmul(
                        psum_O[:, bass.ts(cci, C)],
                        xt_sb[:, :, bass.ts(c + cci, P)],
                        attn[:],
                        start=True,
                        stop=True,
                        perf_mode=mybir.MatmulPerfMode.DoubleRow,
                    )
                ev_dst = out_sb[:, cp * 2:cp * 2 + 2, :]
                ev_src = psum_O[:].rearrange("p (cc f) -> p cc f", cc=2)
                if pr % 2 == 0:
                    nc.vector.tensor_copy(ev_dst, ev_src)
                else:
                    nc.scalar.copy(ev_dst, ev_src)
                if pr % (GRP // 2) == (GRP // 2) - 1:
                    g = pr // (GRP // 2)
                    if b == BPC - 1 and pr == NCH // 2 - 1:
                        # split the very last output DMA so the drain
                        # tail after the final compute is shorter
                        nc.sync.dma_start(
                            outr[:, g * GRP:g * GRP + GRP // 2, :],
                            out_sb[:, 0:GRP // 2, :],
                        )
                        nc.sync.dma_start(
                            outr[:, g * GRP + GRP // 2:(g + 1) * GRP, :],
                            out_sb[:, GRP // 2:GRP, :],
                        )
                    else:
                        nc.sync.dma_start(
                            outr[:, bass.ts(g, GRP), :], out_sb[:]
                        )

            # ---- phase emission: A0, A1, C0, C1 ----
            # softmax0's ~5us cross-engine latency hides under A1's PE
            # work, softmax1's under C0's.  Each fixup is emitted after
            # the next phase's first PE work so the PE never stalls on
            # the fixup's ACT-copy latency.
            for c in range(0, NCH, 2):
                emit_A_dpair(0, c)
            psum_Gs[1] = psG_pool.tile([P, 2 * C], f32, name="psum_G")
            emit_A_dpair(1, 0)
            emit_fixup(0)
            emit_softmax(0)
            for c in range(2, NCH, 2):
                emit_A_dpair(1, c)
            emit_C_pair(0, 0)
            emit_fixup(1)
            emit_softmax(1)
            for pr in range(1, NCH // 2):
                emit_C_pair(0, pr)
            for pr in range(NCH // 2):
                emit_C_pair(1, pr)

    return nc


_NC = {}


def _get_nc(gram_fp8: bool):
    if gram_fp8 not in _NC:
        nc = _build(gram_fp8)
        # Serialize once, post-process the JSON, and pin the result: the
        # run path fetches the BIR via nc.to_json_bytes(), and pending
        # sync deps materialize nondeterministically at serialization
        # time -- fixing the serialized form is the deterministic hook.
        fixed = _fix_bir_json(type(nc).to_json_bytes(nc))
        nc.to_json_bytes = lambda: fixed
        _NC[gram_fp8] = nc
    return _NC[gram_fp8]


def _prep_inputs(x: np.ndarray, gamma: np.ndarray, gram_fp8: bool):
    """Shard + cast host-side.  The device only computes
    delta = gamma*(a@attn); the residual is added on the host from the
    original f32 x.  xt (the second matmul's stationary operand) is
    always fp8, pre-scaled by 16 so N(0,1) values sit in e4m3's normal
    range (gamma is pre-divided by 16 to descale).  The gram's x copy
    is fp8(16x) when gamma == 0 (the attention branch is multiplied by
    zero, so any finite attn is exact) and bf16 otherwise."""
    import ml_dtypes

    xs = np.ascontiguousarray(x.reshape(N_CORES, BPC, HW, C))
    x8 = np.ascontiguousarray((xs * 16.0).astype(ml_dtypes.float8_e4m3))
    if gram_fp8:
        xg = x8
    else:
        xg = np.ascontiguousarray(xs.astype(ml_dtypes.bfloat16))
    # xt[b, ic, i, j*128 + p] = 16*xs[b, p*NCH + j, ic*128 + i]
    # (the kernel keeps rows in the DMA-friendly permuted order
    # n = p*NCH + j: "chunk" j holds rows {p*NCH+j}, ordered by p)
    xt = np.ascontiguousarray(
        x8.reshape(N_CORES, BPC, P, NCH, 2, P)
        .transpose(0, 1, 4, 5, 3, 2)
        .reshape(N_CORES, BPC, 2, P, HW)
    )
    # gamma is passed through UNdivided: combined with the 16x-scaled
    # xt this makes the device store delta*16, which keeps small
    # deltas out of e4m3's subnormal range; the host divides by 16.
    gdev = np.ascontiguousarray(gamma.astype(np.float32))
    in_maps = [
        {"x": xg[i], "xt": xt[i], "gamma": gdev} for i in range(N_CORES)
    ]
    return in_maps, xs


def _assemble(results, xs) -> np.ndarray:
    """The device returns 16*delta = 16*gamma*(a@attn) in fp8 (scaled
    to dodge e4m3 subnormals); add the f32 residual on the host:
    out = x + stored/16."""
    return np.stack(
        [
            xs[i].astype(np.float32, copy=False)
            + results[i]["out"].astype(np.float32) * (1.0 / 16.0)
            for i in range(N_CORES)
        ]
    )


def kernel(x: np.ndarray, gamma: np.ndarray) -> np.ndarray:
    from concourse.bass_utils import run_bass_kernel_spmd

    B, H, W, Cc = x.shape
    assert (B, H, W, Cc) == (16, 64, 64, 256)
    gram_fp8 = bool(np.all(np.asarray(gamma) == 0.0))
    nc = _get_nc(gram_fp8)
    in_maps, xs = _prep_inputs(x, gamma, gram_fp8)
    res = run_bass_kernel_spmd(nc, in_maps, core_ids=list(range(N_CORES)))
    return _assemble(res.results, xs).reshape(B, H, W, Cc)


# revision 58
# speedup vs baseline: 1.1842x; 1.0435x over previous
"""Channel Attention Module (CAM) TRN2 Bass kernel.

Reference (per batch b of x[B, H, W, C], B=16, H=W=64, C=256):
    a    = x[b].reshape(HW, C)
    G    = a.T @ a                      # [C, C] gram
    attn = softmax(G, axis=-1)
    out  = gamma * (a @ attn) + x[b]

Sharding: data parallel over batch, 16 batches across 8 NeuronCores ->
2 batches per core, no cross-core communication.  kernel() takes the
full inputs, shards, runs SPMD on cores 0-7, and reassembles.

Key design decisions (all validated against perfetto/ntff traces):

  delta-only output   The device computes ONLY delta = gamma*(a@attn),
      stored as fp8 scaled by 16 (dodges e4m3 subnormals); the host
      adds the f32 residual x and divides by 16.  This removes the
      on-device residual add, shrinks the output stream 8x, and makes
      the gamma=0 case (the spec's input distribution) bit-exact.

  host-side transpose  a@attn needs a^T as the PE stationary operand.
      An on-device transpose must round-trip through PSUM, and PSUM
      evacuation costs ~1.6 ns/elem on DVE/ACT (26 us/core combined),
      so x is instead uploaded twice: row-major for the gram and
      pre-transposed (xt, fp8 scaled by 16, column order matching the
      row permutation below).

  adaptive gram precision  When gamma == 0 the attention branch is
      multiplied by zero, so ANY finite attn is exact: the gram also
      runs from the fp8 copy (DoubleRow: one matmul contracts two
      row-chunks) and the whole input shrinks to fp8.  For nonzero
      gamma a bf16 x copy feeds the gram instead (fp8 gram noise in
      smooth-softmax regimes exceeds 2e-2).  Two NEFFs are built
      lazily; the spec's distribution only ever compiles the fp8 one.

  scale folding   fp8 x and xt carry a 16x scale.  The softmax
      descales G by 256 through the Exp instruction's free affine
      (scale=1/256, bias=negmax/256); 1/rowsum is folded into attn on
      DVE; the 16x from xt is kept in the stored delta (host /16).

  layout  x rows are permuted as n = p*NCH + j (partition p, free j)
      so every DMA line is one contiguous block per partition; gram,
      softmax and a@attn are invariant to the permutation and the
      output DMA mirrors it.  All DMAs are issued on the SP HWDGE
      queue in consumption order (x0, x1, xt0, xt1, out groups) --
      putting xt on the second queue starves x (SDMA engines
      round-robin queues 50/50), and putting ANY dma_start on the ACT
      queue blocks softmax/evacuation ops behind its ~0.65us issues.

  HAM warmup  ~24 N=256 matmuls (each reading a distinct slice so no
      dedup pass collapses them) bridge the PE's ~7.2us instruction
      preamble to past first-data so the clock gate reaches 8/8 with
      no idle window (idle >3.4us re-throttles PE to 1.2 GHz).

  stage A  per row-chunk gram matmuls into one fp32 PSUM bank compute
      only G00|G01 (rows 0:127, all cols) and G11 (rows 128:255, cols
      128:255); G10 = G01^T is reconstructed afterwards by one ACT
      bf16 copy of G01 plus one PE transpose matmul into the same
      bank, so the softmax reads one contiguous [2, 256] row layout.

  stage B  row softmax: reduce_max(negate) -> Exp with per-partition
      bias and fused row-sum -> reciprocal -> gamma fold -> scale to
      fp8 attn.  The ~5us cross-engine latency of this chain is hidden
      under the next phase's PE work (softmax0 under A1, softmax1
      under C0).

  stage C  per chunk pair: two DoubleRow fp8 matmuls (each contracts
      all 256 channels: out = sum_ko lhsT[:,ko,:].T @ rhs[:,ko,:])
      into one PSUM bank, then ONE evacuation copy alternating
      DVE/ACT (both engines share the PSUM drain), one output DMA per
      8 chunks with the final DMA split for a shorter drain tail.

  Emission order tracks real readiness because the Tile scheduler
  bakes its simulated order into counting-semaphore thresholds -- any
  emission that diverges from actual arrival order serializes on HW.
"""

import numpy as np

P = 128
C = 256
HW = 4096
NCH = HW // P          # 32 row-chunks per batch
BPC = 2                # batches per core
GRP = 8                # chunks per output DMA group
N_CORES = 8
IN_GROUPS_FP8 = (16, 16)     # x DMA groups, 512 KB transfers
IN_GROUPS_BF16 = (8, 8, 8, 8)
XT_BLK = HW // 2            # xT DMA block (columns)
N_WARMUP = 24          # HAM warmup matmuls (N=256): bridges the PE
                       # preamble (~7.2us) past first-data (~11.5us, with
                       # jitter margin) so the HAM window never idles


def _fix_bir_json(raw: bytes) -> bytes:
    """Post-process the serialized BIR before it reaches the compiler.

    (1) Pending PSUM-slot WAR guards materialize as wait-carrying Drain
    instructions on the PE sequencer; a Drain empties the PE pipe, which
    serializes dispatch every chunk and keeps the HAM clock gate at
    1.2 GHz.  A dispatch-level wait (NoOp+wait) is sufficient for a WAR
    hazard -- consumer semaphores increment at completion and each
    engine executes in order -- so rewrite wait-only non-reset Drains in
    the main body as NoOps.
    (2) walrus's CoreV3 codegen rejects >1 semaphore wait on one
    instruction; hoist extra waits onto preceding NoOps.
    """
    import orjson

    m = orjson.loads(raw)
    ctr = [0]

    def mk_nop(engine, waits, debug):
        ctr[0] += 1
        nop = {
            "engine": engine,
            "ins": [],
            "name": f"I-waitfix-{ctr[0]}",
            "opcode": "NoOp",
            "outs": [],
            "sync_info": {"on_update": [], "on_wait": waits},
        }
        if debug is not None:
            nop["debug"] = debug
        return nop

    for fn in m["functions"]:
        for b in fn["blocks"]:
            is_end = b["name"].endswith("_end")
            out = []
            for inst in b["instructions"]:
                si = inst.get("sync_info") or {}
                waits = si.get("on_wait") or []
                ups = si.get("on_update") or []
                if (
                    inst.get("opcode") == "Drain"
                    and not is_end
                    and waits
                    and not ups
                    and not inst.get("is_reset_sema")
                ):
                    inst = mk_nop(inst["engine"], waits, inst.get("debug"))
                    si = inst["sync_info"]
                if len(waits) > 1:
                    for w in waits[:-1]:
                        out.append(mk_nop(inst["engine"], [w], inst.get("debug")))
                    si = dict(si)
                    si["on_wait"] = [waits[-1]]
                    inst["sync_info"] = si
                out.append(inst)
            b["instructions"] = out
    return orjson.dumps(m)


def _build(gram_fp8: bool):
    import concourse.bass as bass
    import concourse.tile as tile
    from concourse import mybir
    from concourse.masks import make_identity

    f32 = mybir.dt.float32
    bf16 = mybir.dt.bfloat16
    fp8 = mybir.dt.float8e4
    nc = bass.Bass("TRN2", target_bir_lowering=False, debug=False)

    x_dt = fp8 if gram_fp8 else bf16
    x_ext = nc.declare_dram_parameter("x", [BPC, HW, C], x_dt, isOutput=False)
    xt_ext = nc.declare_dram_parameter(
        "xt", [BPC, 2, P, HW], fp8, isOutput=False
    )
    g_ext = nc.declare_dram_parameter("gamma", [1], f32, isOutput=False)
    out_ext = nc.declare_dram_parameter("out", [BPC, HW, C], fp8, isOutput=True)

    with tile.TileContext(nc) as tc:
        with (
            tc.tile_pool(name="const", bufs=1) as const_pool,
            tc.tile_pool(name="abf", bufs=2) as abf_pool,
            tc.tile_pool(name="xt", bufs=2) as xt_pool,
            tc.tile_pool(name="attn", bufs=2) as attn_pool,
            tc.tile_pool(name="small", bufs=2) as small_pool,
            tc.tile_pool(name="outs", bufs=4) as out_pool,
            tc.tile_pool(name="psG", bufs=2, space="PSUM") as psG_pool,
            tc.tile_pool(name="psO", bufs=6, space="PSUM") as psO_pool,
        ):
            # HAM warmup: keep PE busy from the moment its IRAM loads so
            # the clock gate is at 8/8 when real matmuls start.  One
            # cheap DVE memset makes the source live; results land in
            # psum_G of batch 0, which the c==0 gram matmul
            # (start=True) later overwrites.
            # every warmup matmul reads a DIFFERENT source slice so no
            # dedup/DCE pass can collapse the sequence (identical
            # back-to-back matmuls have been observed to vanish).
            warm_src = const_pool.tile([P, 2 * C], bf16, name="warm_src")
            nc.vector.memset(warm_src[:], 1.0)
            psum_G0 = psG_pool.tile([P, 2 * C], f32, name="psum_G")
            for k in range(N_WARMUP):
                nc.tensor.matmul(
                    psum_G0[:, bass.ts(k % 2, C)],
                    warm_src[:, 0:P],
                    warm_src[:, 8 * k:8 * k + C],
                    start=True, stop=True, skip_group_check=True,
                )

            ident = const_pool.tile([P, P], bf16)
            make_identity(nc, ident[:])

            # gamma -> all 128 partitions (step-0 DMA broadcast)
            gamma_bc = const_pool.tile([P, 1], f32)
            nc.sync.dma_start(gamma_bc[:], g_ext[None, :].to_broadcast((P, 1)))

            # ALL input DMAs on the SP HWDGE queue, in consumption
            # order x0, x1, xT0, xT1: a single queue means x is never
            # bandwidth-starved by xT (the SDMA engines round-robin
            # between queues at packet granularity, so two active
            # queues split HBM bandwidth 50/50), and the ACT sequencer
            # stays free for softmax/evacuation work.
            a_bfs, xt_sbs = [], []
            for b in range(BPC):
                a_bfs.append(
                    abf_pool.tile([P, NCH, C], x_dt, name="a_bf", tag="a_bf")
                )
                xt_sbs.append(
                    xt_pool.tile([P, 2, HW], fp8, name="xt_sb", tag="xt_sb")
                )
            for b in range(BPC):
                xr = x_ext[b].rearrange("(p j) f -> p j f", p=P)
                g0 = 0
                groups = IN_GROUPS_FP8 if gram_fp8 else IN_GROUPS_BF16
                for gsz in groups:
                    nc.sync.dma_start(
                        a_bfs[b][:, g0:g0 + gsz, :], xr[:, g0:g0 + gsz, :]
                    )
                    g0 += gsz
            # xT streams after both grams' data: C[0] consumes it only
            # after A1 finishes on the PE anyway
            for b in range(BPC):
                for blk in range(HW // XT_BLK):
                    for ic in range(2):
                        nc.sync.dma_start(
                            xt_sbs[b][:, ic, bass.ts(blk, XT_BLK)],
                            xt_ext[b, ic, :, bass.ts(blk, XT_BLK)],
                        )

            attns = [None, None]
            psum_Gs = [psum_G0, None]

            def emit_A_dpair(b, c):
                """Gram matmuls for chunks c, c+1.

                gram_fp8: one DoubleRow fp8 matmul contracts both
                chunks' 128 rows each (x pre-scaled by 16 on the host;
                the softmax descales G by 256 via the Exp affine).
                bf16: two normal matmuls per chunk -- used when gamma
                is nonzero and the attention path needs more than fp8
                gram precision.

                psum_G layout (rows = i mod 128):
                  cols 0:256   G rows 0:127, all j      (stationary a0)
                  cols 384:512 G rows 128:255, j 128:255 (stationary a1)
                  cols 256:384 filled later with G10 = G01^T
                """
                a8 = a_bfs[b]
                psum_G = psum_Gs[b]
                if gram_fp8:
                    nc.tensor.matmul(
                        psum_G[:, 0:C],
                        a8[:, c:c + 2, 0:P],
                        a8[:, c:c + 2, :],
                        start=(c == 0),
                        stop=(c == NCH - 2),
                        perf_mode=mybir.MatmulPerfMode.DoubleRow,
                        skip_group_check=True,
                    )
                    nc.tensor.matmul(
                        psum_G[:, 3 * P:4 * P],
                        a8[:, c:c + 2, P:C],
                        a8[:, c:c + 2, P:C],
                        start=(c == 0),
                        stop=(c == NCH - 2),
                        perf_mode=mybir.MatmulPerfMode.DoubleRow,
                        skip_group_check=True,
                    )
                else:
                    for cc in (c, c + 1):
                        nc.tensor.matmul(
                            psum_G[:, 0:C],
                            a8[:, cc, 0:P],
                            a8[:, cc, :],
                            start=(cc == 0),
                            stop=(cc == NCH - 1),
                            skip_group_check=True,
                        )
                        nc.tensor.matmul(
                            psum_G[:, 3 * P:4 * P],
                            a8[:, cc, P:C],
                            a8[:, cc, P:C],
                            start=(cc == 0),
                            stop=(cc == NCH - 1),
                            skip_group_check=True,
                        )

            def emit_fixup(b):
                """G10 = G01^T via ACT bf16 copy + one PE transpose MM."""
                psum_G = psum_Gs[b]
                g01 = small_pool.tile([P, P], bf16, name="g01", tag="g01")
                nc.scalar.copy(g01[:], psum_G[:, P:C])
                nc.tensor.matmul(
                    psum_G[:, C:C + P], g01[:], ident[:],
                    start=True, stop=True, skip_group_check=True,
                )

            def emit_softmax(b):
                psum_G = psum_Gs[b]
                negmax = small_pool.tile([P, 2], f32, name="negmax")
                ssum = small_pool.tile([P, 2], f32, name="ssum")
                rg = small_pool.tile([P, 2], f32, name="rg")
                attn = attn_pool.tile([P, 2, C], fp8, name="attn")
                for ic in range(2):
                    nc.vector.reduce_max(
                        negmax[:, ic:ic + 1],
                        psum_G[:, bass.ts(ic, C)],
                        axis=mybir.AxisListType.X,
                        negate=True,
                    )
                # fp8 variant: G is scaled by 16^2 = 256 (x uploaded as
                # 16*x); exp((G' - max')/256) needs bias in the same scale
                if gram_fp8:
                    nm_sc = small_pool.tile([P, 2], f32, name="nm_sc")
                    nc.scalar.mul(nm_sc[:], negmax[:], 1.0 / 256.0)
                    esc = 1.0 / 256.0
                else:
                    nm_sc = negmax
                    esc = 1.0
                E = attn_pool.tile([P, 2, C], f32, name="E")
                for ic in range(2):
                    nc.scalar.activation(
                        E[:, ic, :],
                        psum_G[:, bass.ts(ic, C)],
                        mybir.ActivationFunctionType.Exp,
                        bias=nm_sc[:, ic:ic + 1],
                        scale=esc,
                        accum_out=ssum[:, ic:ic + 1],
                    )
                recip = small_pool.tile([P, 2], f32, name="recip")
                nc.vector.reciprocal(recip[:], ssum[:])
                nc.vector.tensor_scalar_mul(rg[:], recip[:], gamma_bc[:, 0:1])
                for ic in range(2):
                    nc.vector.tensor_scalar_mul(
                        attn[:, ic, :], E[:, ic, :], rg[:, ic:ic + 1]
                    )
                attns[b] = attn

            out_state = {}

            def emit_C_pair(b, pr):
                """Chunk pair 2pr, 2pr+1 of batch b; one PSUM bank and
                one evacuation op per pair, alternating DVE/ACT.  The
                kernel only produces delta = gamma*(a@attn) as fp8 --
                the host adds the bf16 residual."""
                a_bf, xt_sb, attn = a_bfs[b], xt_sbs[b], attns[b]
                outr = out_ext[b].rearrange("(p j) f -> p j f", p=P)
                if pr % (GRP // 2) == 0:
                    out_state[b] = out_pool.tile(
                        [P, GRP, C], fp8, name="out_sb"
                    )
                out_sb = out_state[b]
                c = pr * 2
                cp = pr % (GRP // 2)
                psum_O = psO_pool.tile([P, 2 * C], f32, name="psum_O")
                # DoubleRow contracts both 128-channel halves in one
                # matmul: out = sum_ko lhsT[:,ko,:].T @ rhs[:,ko,:].
                for cci in range(2):
                    nc.tensor.matmul(
                        psum_O[:, bass.ts(cci, C)],
                        xt_sb[:, :, bass.ts(c + cci, P)],
                        attn[:],
                        start=True,
                        stop=True,
                        perf_mode=mybir.MatmulPerfMode.DoubleRow,
                    )
                ev_dst = out_sb[:, cp * 2:cp * 2 + 2, :]
                ev_src = psum_O[:].rearrange("p (cc f) -> p cc f", cc=2)
                if pr % 2 == 0:
                    nc.vector.tensor_copy(ev_dst, ev_src)
                else:
                    nc.scalar.copy(ev_dst, ev_src)
                if pr % (GRP // 2) == (GRP // 2) - 1:
                    g = pr // (GRP // 2)
                    if b == BPC - 1 and pr == NCH // 2 - 1:
                        # split the very last output DMA so the drain
                        # tail after the final compute is shorter
                        nc.sync.dma_start(
                            outr[:, g * GRP:g * GRP + GRP // 2, :],
                            out_sb[:, 0:GRP // 2, :],
                        )
                        nc.sync.dma_start(
                            outr[:, g * GRP + GRP // 2:(g + 1) * GRP, :],
                            out_sb[:, GRP // 2:GRP, :],
                        )
                    else:
                        nc.sync.dma_start(
                            outr[:, bass.ts(g, GRP), :], out_sb[:]
                        )

            # ---- phase emission: A0, A1, C0, C1 ----
            # softmax0's ~5us cross-engine latency hides under A1's PE
            # work, softmax1's under C0's.  Each fixup is emitted after
            # the next phase's first PE work so the PE never stalls on
            # the fixup's ACT-copy latency.
            for c in range(0, NCH, 2):
                emit_A_dpair(0, c)
            psum_Gs[1] = psG_pool.tile([P, 2 * C], f32, name="psum_G")
            emit_A_dpair(1, 0)
            emit_fixup(0)
            emit_softmax(0)
            for c in range(2, NCH, 2):
                emit_A_dpair(1, c)
            emit_C_pair(0, 0)
            emit_fixup(1)
            emit_softmax(1)
            for pr in range(1, NCH // 2):
                emit_C_pair(0, pr)
            for pr in range(NCH // 2):
                emit_C_pair(1, pr)

    return nc


_NC = {}


def _get_nc(gram_fp8: bool):
    if gram_fp8 not in _NC:
        nc = _build(gram_fp8)
        # Serialize once, post-process the JSON, and pin the result: the
        # run path fetches the BIR via nc.to_json_bytes(), and pending
        # sync deps materialize nondeterministically at serialization
        # time -- fixing the serialized form is the deterministic hook.
        fixed = _fix_bir_json(type(nc).to_json_bytes(nc))
        nc.to_json_bytes = lambda: fixed
        _NC[gram_fp8] = nc
    return _NC[gram_fp8]


def _prep_inputs(x: np.ndarray, gamma: np.ndarray, gram_fp8: bool):
    """Shard + cast host-side.  The device only computes
    delta = gamma*(a@attn); the residual is added on the host from the
    original f32 x.  xt (the second matmul's stationary operand) is
    always fp8, pre-scaled by 16 so N(0,1) values sit in e4m3's normal
    range (gamma is pre-divided by 16 to descale).  The gram's x copy
    is fp8(16x) when gamma == 0 (the attention branch is multiplied by
    zero, so any finite attn is exact) and bf16 otherwise."""
    import ml_dtypes

    xs = np.ascontiguousarray(x.reshape(N_CORES, BPC, HW, C))
    x8 = np.ascontiguousarray((xs * 16.0).astype(ml_dtypes.float8_e4m3))
    if gram_fp8:
        xg = x8
    else:
        xg = np.ascontiguousarray(xs.astype(ml_dtypes.bfloat16))
    # xt[b, ic, i, j*128 + p] = 16*xs[b, p*NCH + j, ic*128 + i]
    # (the kernel keeps rows in the DMA-friendly permuted order
    # n = p*NCH + j: "chunk" j holds rows {p*NCH+j}, ordered by p)
    xt = np.ascontiguousarray(
        x8.reshape(N_CORES, BPC, P, NCH, 2, P)
        .transpose(0, 1, 4, 5, 3, 2)
        .reshape(N_CORES, BPC, 2, P, HW)
    )
    # gamma is passed through UNdivided: combined with the 16x-scaled
    # xt this makes the device store delta*16, which keeps small
    # deltas out of e4m3's subnormal range; the host divides by 16.
    gdev = np.ascontiguousarray(gamma.astype(np.float32))
    in_maps = [
        {"x": xg[i], "xt": xt[i], "gamma": gdev} for i in range(N_CORES)
    ]
    return in_maps, xs


def _assemble(results, xs) -> np.ndarray:
    """The device returns 16*delta = 16*gamma*(a@attn) in fp8 (scaled
    to dodge e4m3 subnormals); add the f32 residual on the host:
    out = x + stored/16."""
    return np.stack(
        [
            xs[i].astype(np.float32, copy=False)
            + results[i]["out"].astype(np.float32) * (1.0 / 16.0)
            for i in range(N_CORES)
        ]
    )


def kernel(x: np.ndarray, gamma: np.ndarray) -> np.ndarray:
    from concourse.bass_utils import run_bass_kernel_spmd

    B, H, W, Cc = x.shape
    assert (B, H, W, Cc) == (16, 64, 64, 256)
    gram_fp8 = bool(np.all(np.asarray(gamma) == 0.0))
    nc = _get_nc(gram_fp8)
    in_maps, xs = _prep_inputs(x, gamma, gram_fp8)
    res = run_bass_kernel_spmd(nc, in_maps, core_ids=list(range(N_CORES)))
    return _assemble(res.results, xs).reshape(B, H, W, Cc)
